# revision 19
# baseline (speedup 1.0000x reference)
"""CARAFE (content-aware upsample) + SE kernel for 8 TRN2 NeuronCores.

Sharding: 8 cores = 4 batches x 2 vertical halves. Each core redundantly
computes the kernel-prediction branch (1x1 conv -> 3x3 -> 3x3, BN folded
host-side) for the FULL image of its batch (the SE global mean then needs no
collective), and gathers/upsamples only its 64-row half.

SPMD uniformity: odd cores get a vertically flipped X, ky-flipped conv
weights, and a pixel-shuffle-aware output-channel permutation of enc2/SE
params, so all 8 cores run the identical program on rows 0..63 of their own
domain; the host flips the odd halves back.

The spatial-attention branch cancels exactly against the per-pixel L1
normalization (it scales all 100 channels of a pixel by the same positive
factor) and is skipped. The SE gate is per-channel and is computed on-device.

Gather (SCALE=2, K_UP=5):
  out[c, 2y+dy, 2x+dx] = sum_{i,j} Wn[4*(5i+j)+2dy+dx, y, x] * X[c, y+i-2, x+j-2]
run on VectorE in pixel-partition layout (partition=x, free=c) with fused
scalar_tensor_tensor MACs; per-pixel weights enter as per-partition scalars.

Host<->device transport: X ships as bf16 (the kernel computes in bf16
anyway) and the output returns as bf16 (the gather accumulator is bf16, so
no precision is lost). The jax.jit(shard_map(bass_exec)) executables are
built once and reused across calls (the stock run_bass_kernel_spmd re-traces
and re-ships donated zero output buffers every call), with donated output
buffers created on-device. kernel() streams the 8 cores as CARAFE_GROUPS
(default 4) independent programs so one group's output download overlaps the
next group's input upload on the full-duplex axon tunnel.
"""

import os
from contextlib import ExitStack

import numpy as np
import ml_dtypes

import concourse.bass as bass
import concourse.bacc as bacc
import concourse.tile as tile
from concourse import mybir
from concourse import bass2jax

F32 = mybir.dt.float32
BF16 = mybir.dt.bfloat16
ALU = mybir.AluOpType
ACTF = mybir.ActivationFunctionType
AX = mybir.AxisListType

H = 128
WID = 128
C = 256
CMID = 64
KU2 = 100
N_OWN = int(os.environ.get("CARAFE_ROWS", "64"))
RS = 132          # x-padded row stride for 3x3 conv inputs
N_CORES = 8

_CACHE = {}
_STATE = {}


def _rows(ap2d, off, nrows, width, stride=RS):
    """[p, F] -> [p, nrows, width] view with row stride `stride` at `off`."""
    v = ap2d[:, off:off + nrows * stride]
    return v.rearrange("p (r x) -> p r x", r=nrows, x=stride)[:, :, 0:width]


def _kernel(ctx, tc, n_own, d):
    nc = tc.nc

    wpool = ctx.enter_context(tc.tile_pool(name="weights", bufs=1))
    xtp = ctx.enter_context(tc.tile_pool(name="xt", bufs=1))
    psum = ctx.enter_context(tc.tile_pool(name="psum", bufs=2, space="PSUM"))
    trps = ctx.enter_context(tc.tile_pool(name="trps", bufs=2, space="PSUM"))
    shps = ctx.enter_context(tc.tile_pool(name="shps", bufs=2, space="PSUM"))

    # ---- params ----
    ident = wpool.tile([128, 128], BF16, tag="ident")
    nc.sync.dma_start(ident[:], d["ident"])
    sh_sb = wpool.tile([128, 4 * 128], BF16, tag="sh_sb")
    nc.sync.dma_start(sh_sb[:], d["shifts"])

    with tc.tile_pool(name="wstage", bufs=2) as wst:
        cwt = wpool.tile([128, 2 * CMID], BF16, tag="cwt")
        for ct in range(2):
            st = wst.tile([128, CMID], F32, tag="wstage")
            nc.sync.dma_start(st[:], d["cw"][ct])
            nc.vector.tensor_copy(cwt[:, ct * CMID:(ct + 1) * CMID], st[:])
        st = wst.tile([CMID, 900], F32, tag="ewstage")
        nc.sync.dma_start(st[:], d["ew"])
        ewt = wpool.tile([CMID, 900], BF16, tag="ewt")
        nc.vector.tensor_copy(ewt[:], st[:])
        st = wst.tile([KU2, 900], F32, tag="e2wstage")
        nc.sync.dma_start(st[:], d["e2w"])
        e2wt = wpool.tile([KU2, 900], BF16, tag="e2wt")
        nc.vector.tensor_copy(e2wt[:], st[:])

    def smallparam(name, p):
        t = wpool.tile([p, 1], F32, tag=name)
        nc.sync.dma_start(t[:], d[name])
        return t

    b1t = smallparam("b1", CMID)
    b2t = smallparam("b2", KU2)
    b3t = smallparam("b3", KU2)
    sb1t = smallparam("sb1", 6)
    sb2t = smallparam("sb2", KU2)
    sw1t = wpool.tile([KU2, 6], F32, tag="sw1")
    nc.sync.dma_start(sw1t[:], d["sw1"])
    sw2t = wpool.tile([6, KU2], F32, tag="sw2")
    nc.sync.dma_start(sw2t[:], d["sw2"])

    # ---- persistent feature maps ----
    xt = xtp.tile([128, (n_own + 4) * C], BF16, tag="xt")
    nc.vector.memset(xt[:, 0:2 * C], 0.0)          # rows above image = 0

    encp = ctx.enter_context(tc.tile_pool(name="enc", bufs=1))
    _w1cm = tc.tile_pool(name="w1", bufs=1)
    w1p = _w1cm.__enter__()
    w1 = w1p.tile([CMID, 131 * RS], BF16, tag="w1")
    nc.vector.memset(w1[:], 0.0)

    # ---- phase 1: X load (already bf16), XT transposes, conv1x1 ----
    with tc.tile_pool(name="xbf", bufs=1) as xbfp:
        xbf = [xbfp.tile([128, H * WID], BF16, tag=f"xbf{ct}", name=f"xbf{ct}")
               for ct in range(2)]
        CHR = 16
        for ct in range(2):
            for rb in range(0, H, CHR):
                nc.sync.dma_start(
                    xbf[ct][:, rb * WID:(rb + CHR) * WID],
                    d["x"][ct, :, rb * WID:(rb + CHR) * WID])

        for r in range(n_own + 2):
            for ct in range(2):
                tp = trps.tile([128, 128], BF16, tag="trp")
                nc.tensor.transpose(
                    tp[:], xbf[ct][:, r * WID:(r + 1) * WID], ident[:])
                o = (r + 2) * C + ct * 128
                nc.vector.tensor_copy(xt[:, o:o + 128], tp[:])

        for rb in range(0, H, 4):
            ps = psum.tile([128, 512], F32, tag="cps")
            for ct in range(2):
                nc.tensor.matmul(
                    ps[:CMID, :], cwt[:, ct * CMID:(ct + 1) * CMID],
                    xbf[ct][:, rb * WID:(rb + 4) * WID],
                    start=(ct == 0), stop=(ct == 1))
            nc.vector.tensor_scalar(
                _rows(w1, (rb + 1) * RS + 2, 4, WID),
                ps[:CMID, :].rearrange("p (r x) -> p r x", r=4, x=WID),
                b1t[:, 0:1], 0.0, op0=ALU.add, op1=ALU.max)

    # ---- phases 2: the two 3x3 convs ----
    def conv3x3(src, lhsT, bias_t, dst, dst_is_padded):
        for rb in range(0, H, 4):
            ps = psum.tile([128, 512], F32, tag="cps")
            for t in range(9):
                ky, kx = t // 3, t % 3
                rhs = _rows(src, (rb + ky) * RS + 1 + kx, 4, WID)
                nc.tensor.matmul(ps[:KU2, :], lhsT[:, t * KU2:(t + 1) * KU2],
                                 rhs, start=(t == 0), stop=(t == 8))
            if dst_is_padded:
                dv = _rows(dst, (rb + 1) * RS + 2, 4, WID)
            else:
                dv = dst[:, rb * WID:(rb + 4) * WID].rearrange(
                    "p (r x) -> p r x", r=4, x=WID)
            nc.vector.tensor_scalar(
                dv, ps[:KU2, :].rearrange("p (r x) -> p r x", r=4, x=WID),
                bias_t[:, 0:1], None, op0=ALU.add)

    enc = encp.tile([KU2, 131 * RS], BF16, tag="enc")
    nc.gpsimd.memset(enc[:], 0.0)
    conv3x3(w1, ewt, b2t, enc, True)
    _w1cm.__exit__(None, None, None)
    w100p = ctx.enter_context(tc.tile_pool(name="w100", bufs=1))
    w100 = w100p.tile([KU2, H * WID], BF16, tag="w100")
    conv3x3(enc, e2wt, b3t, w100, False)

    # ---- phase 3: SE gate ----
    s_sb = wpool.tile([KU2, 1], F32, tag="s_sb")
    nc.vector.tensor_reduce(s_sb[:], w100[:], axis=AX.X, op=ALU.add)
    ps = psum.tile([128, 512], F32, tag="cps")
    nc.tensor.matmul(ps[:6, 0:1], sw1t[:], s_sb[:], start=True, stop=True)
    h_sb = wpool.tile([6, 1], F32, tag="h_sb")
    nc.vector.tensor_scalar(h_sb[:], ps[:6, 0:1], sb1t[:, 0:1], 0.0,
                            op0=ALU.add, op1=ALU.max)
    ps2 = psum.tile([128, 512], F32, tag="cps")
    nc.tensor.matmul(ps2[:KU2, 0:1], sw2t[:], h_sb[:], start=True, stop=True)
    gate = wpool.tile([KU2, 1], F32, tag="gate")
    nc.scalar.activation(gate[:], ps2[:KU2, 0:1], ACTF.Sigmoid,
                         bias=sb2t[:, 0:1])
    nc.vector.tensor_scalar(w100[:, :n_own * WID], w100[:, :n_own * WID],
                            gate[:, 0:1], 2.0, op0=ALU.mult, op1=ALU.mult)

    # ---- phase 4: transpose W, L1-normalize -> WN ----
    wnp = ctx.enter_context(tc.tile_pool(name="wn", bufs=1))
    wt = wnp.tile([128, n_own * KU2], BF16, tag="wt")
    for y in range(n_own):
        tp = trps.tile([128, 128], BF16, tag="trp")
        nc.tensor.transpose(tp[:, :KU2], w100[:, y * WID:(y + 1) * WID],
                            ident[:KU2, :KU2])
        nc.vector.tensor_copy(wt[:, y * KU2:(y + 1) * KU2], tp[:, :KU2])

    wt3 = wt[:].rearrange("p (y k) -> p y k", y=n_own, k=KU2)
    nrm = wnp.tile([128, 4 * n_own], F32, tag="nrm")
    for sub in range(4):
        nc.vector.tensor_reduce(
            nrm[:, sub * n_own:(sub + 1) * n_own],
            wt3[:, :, sub:KU2:4], axis=AX.X, op=ALU.add,
            apply_absolute_value=True)
    nc.vector.tensor_scalar(nrm[:], nrm[:], 1e-12, None, op0=ALU.max)
    nrmi = wnp.tile([128, 4 * n_own], F32, tag="nrmi")
    nc.vector.reciprocal(nrmi[:], nrm[:])

    wn = wnp.tile([128, n_own * KU2], F32, tag="wnrm")
    wn3 = wn[:].rearrange("p (y k) -> p y k", y=n_own, k=KU2)
    for sub in range(4):
        nc.vector.tensor_tensor(
            wn3[:, :, sub * 25:(sub + 1) * 25],
            wt3[:, :, sub:KU2:4],
            nrmi[:, sub * n_own:(sub + 1) * n_own].unsqueeze(2).broadcast_to(
                [128, n_own, 25]),
            op=ALU.mult)

    # ---- phase 5: gather + upsample + store ----
    # x-shifted copies of XT rows (PE shift-matmul), 6-slot ring buffer.
    # sh_sb column block si holds S_dlt with dlt = (-2,-1,1,2)[si]:
    # out[x] = xtrow[x+dlt], zeros outside the image.
    RING = 6
    xtsp = ctx.enter_context(tc.tile_pool(name="xtsp", bufs=1))
    xts = xtsp.tile([128, RING * 4 * C], BF16, tag="xts")

    def fill_slot(s):
        ps = shps.tile([128, 4 * C], F32, tag="shps", name=f"shp{s}")
        for si in range(4):
            nc.tensor.matmul(ps[:, si * C:(si + 1) * C],
                             sh_sb[:, si * 128:(si + 1) * 128],
                             xt[:, s * C:(s + 1) * C], start=True, stop=True)
        nc.scalar.copy(xts[:, (s % RING) * 4 * C:((s % RING) + 1) * 4 * C],
                       ps[:])

    # The 25-tap MAC chain is DVE-bound (scalar_tensor_tensor has no DVE
    # perf mode: 25 x ~324ns/(y,sub)). Offload 13 of the 20 shifted-tap
    # products to the otherwise-idle Activation engine (activation-Copy
    # with per-partition scale); DVE folds those in with bf16 2x-mode
    # tensor_tensor adds (~191ns) instead of full STT MACs.
    out2 = d["out"].rearrange("c h w -> c (h w)")
    with tc.tile_pool(name="acc", bufs=4) as accp, \
         tc.tile_pool(name="prd", bufs=6) as prdp, \
         tc.tile_pool(name="stage", bufs=6) as stgp:
        for s in range(4):
            fill_slot(s)
        for y in range(n_own):
            if y + 4 <= n_own + 3:
                fill_slot(y + 4)
            for dy in range(2):
                stg = [stgp.tile([128, 2 * WID], BF16, tag=f"stg{ct}", name=f"stg{ct}")
                       for ct in range(2)]
                for dx in range(2):
                    sub = 2 * dy + dx
                    acc = accp.tile([128, C], BF16, tag="accf")
                    for r in range(5):
                        woff = y * KU2 + sub * 25 + 5 * r
                        slot = ((y + r) % RING) * 4 * C
                        xsrc = xt[:, (y + r) * C:(y + r + 1) * C]
                        if r == 0:
                            nc.vector.tensor_scalar(
                                acc[:], xsrc, wn[:, woff + 2:woff + 3], None,
                                op0=ALU.mult)
                        else:
                            nc.vector.scalar_tensor_tensor(
                                acc[:], xsrc, wn[:, woff + 2:woff + 3],
                                acc[:], op0=ALU.mult, op1=ALU.add)
                        for si, j in ((0, 0), (1, 1), (2, 3), (3, 4)):
                            src = xts[:, slot + si * C:slot + (si + 1) * C]
                            wsc = wn[:, woff + j:woff + j + 1]
                            if (r * 4 + si) % 20 < 13:
                                prd = prdp.tile([128, C], BF16, tag="prd")
                                nc.scalar.mul(prd[:], src, wsc)
                                nc.vector.tensor_tensor(
                                    acc[:], acc[:], prd[:], op=ALU.add)
                            else:
                                nc.vector.scalar_tensor_tensor(
                                    acc[:], src, wsc, acc[:],
                                    op0=ALU.mult, op1=ALU.add)
                    for ct in range(2):
                        tp = trps.tile([128, 128], BF16, tag="trp", name="otr")
                        nc.tensor.transpose(
                            tp[:], acc[:, ct * 128:(ct + 1) * 128], ident[:])
                        dst = stg[ct][:].rearrange(
                            "p (x two) -> p x two", x=WID, two=2)[:, :, dx:dx + 1]
                        tsrc = tp[:].unsqueeze(2)
                        nc.scalar.copy(dst, tsrc)
                for ct in range(2):
                    row = 2 * y + dy
                    nc.sync.dma_start(
                        out2[ct * 128:(ct + 1) * 128,
                             row * 2 * WID:(row + 1) * 2 * WID], stg[ct][:])


def _build_nc(n_own):
    nc = bacc.Bacc("TRN2", target_bir_lowering=False, debug=False,
                   num_devices=N_CORES)
    d = {}
    d["x"] = nc.dram_tensor("x", [2, 128, H * WID], BF16,
                            kind="ExternalInput").ap()
    d["cw"] = nc.dram_tensor("cw", [2, 128, CMID], F32,
                             kind="ExternalInput").ap()
    d["ew"] = nc.dram_tensor("ew", [CMID, 900], F32,
                             kind="ExternalInput").ap()
    d["e2w"] = nc.dram_tensor("e2w", [KU2, 900], F32,
                              kind="ExternalInput").ap()
    for nm, p in [("b1", CMID), ("b2", KU2), ("b3", KU2), ("sb1", 6),
                  ("sb2", KU2)]:
        d[nm] = nc.dram_tensor(nm, [p, 1], F32, kind="ExternalInput").ap()
    d["sw1"] = nc.dram_tensor("sw1", [KU2, 6], F32,
                              kind="ExternalInput").ap()
    d["sw2"] = nc.dram_tensor("sw2", [6, KU2], F32,
                              kind="ExternalInput").ap()
    d["ident"] = nc.dram_tensor("ident", [128, 128], BF16,
                                kind="ExternalInput").ap()
    d["shifts"] = nc.dram_tensor("shifts", [128, 4 * 128], BF16,
                                 kind="ExternalInput").ap()
    d["out"] = nc.dram_tensor("out", [C, H, 2 * WID], BF16,
                              kind="ExternalOutput").ap()

    with tile.TileContext(nc, trace_sim=False) as tc:
        with ExitStack() as ctx:
            _kernel(ctx, tc, n_own, d)
    nc.compile()
    return nc


def _shift_mats():
    sh = np.zeros((128, 4 * 128), np.float32)
    for si, dlt in enumerate((-2, -1, 1, 2)):
        for m in range(128):
            k = m + dlt
            if 0 <= k < 128:
                sh[k, si * 128 + m] = 1.0
    return sh.astype(ml_dtypes.bfloat16)


def _host_prep(inputs):
    X = inputs["X"]
    EPS = 1e-5

    def fold(w, bn):
        g, b, m, v = bn
        s = g / np.sqrt(v + EPS)
        return (w * s.reshape(-1, *([1] * (w.ndim - 1)))).astype(np.float32), \
               (b - m * s).astype(np.float32)

    cw, b1 = fold(np.asarray(inputs["comp_w"])[:, :, 0, 0],
                  np.asarray(inputs["comp_bn"]))
    ew, b2 = fold(np.asarray(inputs["enc_w"]), np.asarray(inputs["enc_bn"]))
    e2w, b3 = fold(np.asarray(inputs["enc2_w"]), np.asarray(inputs["enc2_bn"]))
    sw1 = (np.asarray(inputs["se_w1"], np.float64) / (H * WID)).astype(
        np.float32)
    sw2 = np.asarray(inputs["se_w2"], np.float32)
    sb1 = np.asarray(inputs["se_b1"], np.float32)
    sb2 = np.asarray(inputs["se_b2"], np.float32)

    perm = np.zeros(KU2, np.int64)
    for i in range(5):
        for j in range(5):
            for dy in range(2):
                for dx in range(2):
                    ch = 4 * (5 * i + j) + 2 * dy + dx
                    perm[ch] = 4 * (5 * (4 - i) + j) + 2 * (1 - dy) + dx

    def pack(a, cin):
        return np.ascontiguousarray(
            a.transpose(1, 2, 3, 0).reshape(cin, 9 * KU2))

    cwT = np.ascontiguousarray(cw.T.reshape(2, 128, CMID))
    ew_e, e2w_e = pack(ew, CMID), pack(e2w, KU2)
    ew_o = pack(ew[:, :, ::-1, :], CMID)
    e2w_o = pack(e2w[perm][:, :, ::-1, :], KU2)

    # x is stored as an f32 view (flipped for odd cores); the cast to bf16
    # happens in one pass while filling the concatenated transfer buffer.
    Xv = np.asarray(X, np.float32).reshape(len(X), 2, 128, H, WID)

    base = {
        "cw": cwT, "b1": b1.reshape(-1, 1), "b2": b2.reshape(-1, 1),
        "sb1": sb1.reshape(-1, 1),
        "ident": np.eye(128, dtype=ml_dtypes.bfloat16),
        "shifts": _shift_mats(),
    }
    maps = []
    for k in range(N_CORES):
        b, h = k // 2, k % 2
        m = dict(base)
        if h == 0:
            m["x"] = Xv[b]
            m["ew"], m["e2w"] = ew_e, e2w_e
            m["b3"] = b3.reshape(-1, 1)
            m["sw1"], m["sw2"] = sw1.T, sw2.T
            m["sb2"] = sb2.reshape(-1, 1)
        else:
            m["x"] = Xv[b][:, :, ::-1, :]
            m["ew"], m["e2w"] = ew_o, e2w_o
            m["b3"] = b3[perm].reshape(-1, 1)
            m["sw1"], m["sw2"] = sw1[:, perm].T, sw2[perm, :].T
            m["sb2"] = sb2[perm].reshape(-1, 1)
        maps.append({k2: (v if k2 == "x" else np.ascontiguousarray(v))
                     for k2, v in m.items()})
    return maps


def _build_state(n_own):
    """Build the bass module once and wrap it in a cached jax.jit(shard_map)
    executable (mirrors bass_utils.run_bass_kernel_spmd's axon path, minus
    the per-call retrace)."""
    import jax
    import jax.numpy as jnp
    from jax.sharding import Mesh, PartitionSpec, NamedSharding
    from jax.experimental.shard_map import shard_map

    if n_own not in _CACHE:
        _CACHE[n_own] = _build_nc(n_own)
    nc = _CACHE[n_own]

    bass2jax.install_neuronx_cc_hook()
    partition_name = (nc.partition_id_tensor.name
                      if nc.partition_id_tensor else None)
    in_names, out_names, out_avals, out_shapes = [], [], [], []
    for alloc in nc.m.functions[0].allocations:
        if not isinstance(alloc, mybir.MemoryLocationSet):
            continue
        name = alloc.memorylocations[0].name
        if alloc.kind == "ExternalInput":
            if name != partition_name:
                in_names.append(name)
        elif alloc.kind == "ExternalOutput":
            shape = tuple(alloc.tensor_shape)
            dtype = mybir.dt.np(alloc.dtype)
            out_avals.append(jax.core.ShapedArray(shape, dtype))
            out_names.append(name)
            out_shapes.append((shape, dtype))
    n_params = len(in_names)
    n_outs = len(out_avals)
    all_in_names = list(in_names) + list(out_names)
    if partition_name is not None:
        all_in_names.append(partition_name)
    donate = tuple(range(n_params, n_params + n_outs))

    def _body(*args):
        operands = list(args)
        if partition_name is not None:
            operands.append(bass2jax.partition_id_tensor())
        outs = bass2jax._bass_exec_p.bind(
            *operands, out_avals=tuple(out_avals),
            in_names=tuple(all_in_names), out_names=tuple(out_names),
            lowering_input_output_aliases=(),
            sim_require_finite=True, sim_require_nnan=True, nc=nc)
        return tuple(outs)

    devices = jax.devices()[:N_CORES]
    assert len(devices) == N_CORES, (
        f"need {N_CORES} devices, found {len(jax.devices())}")
    mesh = Mesh(np.asarray(devices), ("core",))
    sharding = NamedSharding(mesh, PartitionSpec("core"))
    in_specs = (PartitionSpec("core"),) * (n_params + n_outs)
    out_specs = (PartitionSpec("core"),) * n_outs
    sharded = jax.jit(
        shard_map(_body, mesh=mesh, in_specs=in_specs, out_specs=out_specs,
                  check_rep=False),
        donate_argnums=donate, keep_unused=True)

    @jax.jit
    def make_zeros():
        return tuple(
            jax.lax.with_sharding_constraint(
                jnp.zeros((N_CORES * s[0], *s[1:]), dt), sharding)
            for s, dt in out_shapes)

    # smaller per-group programs used by kernel() to stream groups through
    # the full-duplex axon tunnel (ship group g+1's input while group g's
    # output downloads)
    n_groups = max(1, min(N_CORES, int(os.environ.get("CARAFE_GROUPS", "4"))))
    while N_CORES % n_groups:
        n_groups -= 1
    ng = N_CORES // n_groups
    groups = []
    for g in range(n_groups):
        gdev = devices[g * ng:(g + 1) * ng]
        gmesh = Mesh(np.asarray(gdev), ("core",))
        gshard = NamedSharding(gmesh, PartitionSpec("core"))
        gsharded = jax.jit(
            shard_map(_body, mesh=gmesh,
                      in_specs=(PartitionSpec("core"),) * (n_params + n_outs),
                      out_specs=(PartitionSpec("core"),) * n_outs,
                      check_rep=False),
            donate_argnums=donate, keep_unused=True)

        def gmz(gshard=gshard):
            return tuple(
                jax.lax.with_sharding_constraint(
                    jnp.zeros((ng * s[0], *s[1:]), dt), gshard)
                for s, dt in out_shapes)

        groups.append({"sharded": gsharded, "make_zeros": jax.jit(gmz),
                       "sharding": gshard})

    return {
        "nc": nc, "sharded": sharded, "make_zeros": make_zeros,
        "in_names": in_names, "out_names": out_names,
        "out_shapes": out_shapes, "sharding": sharding, "devices": devices,
        "groups": groups, "ng": ng,
    }


def _get_state(n_own):
    if n_own not in _STATE:
        _STATE[n_own] = _build_state(n_own)
    return _STATE[n_own]


_XBUF = {}


def _concat_inputs(st, maps, buf_key=None):
    ncores = len(maps)
    out = []
    for name in st["in_names"]:
        if name == "x":
            key = (buf_key, ncores)
            xcat = _XBUF.get(key)
            if xcat is None:
                xcat = np.empty((ncores * 2, 128, H * WID),
                                ml_dtypes.bfloat16)
                if buf_key is not None:
                    _XBUF[key] = xcat
            xv = xcat.reshape(ncores, 2, 128, H, WID)
            for k, m in enumerate(maps):
                xv[k] = m["x"]          # flip + f32->bf16 cast in one pass
            out.append(xcat)
        else:
            out.append(np.concatenate([np.asarray(m[name]) for m in maps],
                                      axis=0))
    return out


def _run_device(st, concat_in, zeros=None):
    """Run the cached executable; returns list of per-core output arrays."""
    if zeros is None:
        zeros = st["make_zeros"]()
    out_arrs = st["sharded"](*concat_in, *zeros)
    for o in out_arrs:
        o.block_until_ready()
    return out_arrs


def _assemble_into(out, res, cores, n_own, h_):
    for i, k in enumerate(cores):
        b, h = k // 2, k % 2
        r = np.asarray(res[i])
        if h == 0:
            out[b, :, :2 * n_own, :] = r[:, :2 * n_own, :]
        else:
            out[b, :, 2 * h_ - 2 * n_own:, :] = r[:, :2 * n_own, :][:, ::-1, :]


def _assemble(inputs, res0, n_own):
    b_, c_, h_, w_ = inputs["X"].shape
    out = np.zeros((b_, c_, 2 * h_, 2 * w_), np.float32)
    _assemble_into(out, res0, range(N_CORES), n_own, h_)
    return out


def kernel(**inputs):
    from concurrent.futures import ThreadPoolExecutor

    n_own = N_OWN
    st = _get_state(n_own)
    groups = st["groups"]
    ng = st["ng"]
    zs = [g["make_zeros"]() for g in groups]   # async; overlaps host prep
    maps = _host_prep(inputs)
    b_, c_, hh, ww = inputs["X"].shape
    if 2 * n_own >= hh:
        out = np.empty((b_, c_, 2 * hh, 2 * ww), np.float32)
    else:  # smoke mode: uncomputed rows must stay zero
        out = np.zeros((b_, c_, 2 * hh, 2 * ww), np.float32)
    shape0 = st["out_shapes"][0][0]

    def fetch(out_g, cores):
        r = np.asarray(out_g[0]).reshape(ng, *shape0)
        _assemble_into(out, r, cores, n_own, hh)

    with ThreadPoolExecutor(2) as ex:
        futs = []
        for g, grp in enumerate(groups):
            cores = range(g * ng, (g + 1) * ng)
            concat_g = _concat_inputs(st, maps[g * ng:(g + 1) * ng],
                                      buf_key=f"g{g}")
            out_g = grp["sharded"](*concat_g, *zs[g])   # uplink + enqueue
            futs.append(ex.submit(fetch, out_g, cores))  # downlink in thread
        for f in futs:
            f.result()
    return out
